# revision 8
# baseline (speedup 1.0000x reference)
"""Trainium2 Bass kernel for multi-level bilinear grid interpolation
(embedding_lookup, nn_COOLCHIC_INTERP_ENC).

Strategy (v2):
  - 8 NeuronCores, data-parallel over query points, sharded spatially by
    latitude into 256 bands (8 ranks x 4 passes x 8 gpsimd cores). Each
    band only touches a handful of grid rows per pyramid level, so each
    band's working set is packed into a per-band table resident in SBUF.
  - Tables store the full bilinear 2x2 quad per (row, col) entry as
    4 x int8 (global per-level symmetric quantization) packed in ONE
    f32 word -> a single d=1 ap_gather index fetches a whole quad.
    Quantization error <= absmax/254 (~0.4%), well inside the 2e-2 gate.
  - Gather indices (int16) and lerp fractions (fp16) are precomputed on
    the host in exactly the layouts the engines want:
      * idx in ap_gather's per-core interleaved stream layout
      * fracs in the lerp layout (partition 16k+q owns stream slice
        [q*F,(q+1)*F) of core k)
    so the gather output de-interleave is ONE SBUF->SBUF DMA with 2KB
    contiguous descriptors (every partition of a core holds the full
    replicated stream; we fan out partition q=0 of each core).
  - DVE does the 9-op bilinear lerp with int8 corner operands, fp32
    intermediates (PSUM), fp16 fracs/result. Host de-quantizes.
"""

import sys

sys.path.insert(0, "/opt/trn_rl_repo")

import numpy as np

from concourse import bacc, bass, mybir
import concourse.tile as tile

# ---------------------------------------------------------------- constants
H_GRID, W_GRID, LEVEL, RES = 721, 1440, 8, 0.25
N_RANKS = 8
N_PASSES = 4
N_Q7 = 8
BANDS = N_RANKS * N_PASSES * N_Q7  # 256
BAND_DEG = 180.0 / BANDS  # 0.703125 (exact binary)
F = 512                   # points per partition per batch
NI = 16 * F               # gather stream length per core (= points/core/batch)

# per-level table geometry: CAP rows x WT cols of quad entries.
# a_l = t32 / res_l is an EXACT power-of-2 scaling of t32 = f32(90 - lat),
# and the band is derived from the same t32 via exact integer arithmetic
# (RS = (45*b) >> (l+4)), so the floor always lands inside the band's row
# window and CAP is exactly the max floor-span per band.
CAPS = [4, 3, 2, 2, 2, 2, 2, 2]
WT = [1440, 720, 360, 180, 90, 45, 23, 12]
ENT = [CAPS[l] * WT[l] for l in range(LEVEL)]
BASE = [sum(ENT[:l]) for l in range(LEVEL)]
TE = sum(ENT)  # 15080 quad entries (f32-packed int8x4) per band

F32 = mybir.dt.float32
F16 = mybir.dt.float16
I16 = mybir.dt.int16
I8 = mybir.dt.int8


def _res(l):
    return RES * (2.0 ** l)


# ---------------------------------------------------------------- device kernel
def build_kernel(n_batch):
    """Per-rank SPMD Bass program. c_band = n_batch * NI points per band."""
    nc = bacc.Bacc(None, target_bir_lowering=False)

    tab_t = nc.declare_dram_parameter("tab", [N_PASSES, N_Q7, TE], F32, False)
    meta_t = nc.declare_dram_parameter(
        "meta", [N_PASSES, n_batch, LEVEL, 128, 3, F], I16, False)
    out_t = nc.declare_dram_parameter(
        "out", [N_PASSES, n_batch, LEVEL, 128, F], F16, True)

    sub = mybir.AluOpType.subtract
    add = mybir.AluOpType.add
    mult = mybir.AluOpType.mult

    from contextlib import ExitStack

    with tile.TileContext(nc) as tc, ExitStack() as es:
        ptab = es.enter_context(tc.tile_pool(name="ptab", bufs=2))
        pdst = es.enter_context(tc.tile_pool(name="pdst", bufs=2))
        pm = es.enter_context(tc.tile_pool(name="pm", bufs=2))
        pq = es.enter_context(tc.tile_pool(name="pq", bufs=2))
        pr = es.enter_context(tc.tile_pool(name="pr", bufs=2))
        pt = es.enter_context(tc.tile_pool(name="pt", bufs=2))

        for p in range(N_PASSES):
            tabs = ptab.tile([128, TE], F32, tag="tabs")
            for q in range(16):
                nc.sync.dma_start(out=tabs[q::16], in_=tab_t[p])

            for bi in range(n_batch):
                for l in range(LEVEL):
                    m = pm.tile([128, 3, F], I16, tag="meta")
                    nc.sync.dma_start(out=m[:], in_=meta_t[p, bi, l])

                    dst = pdst.tile([128, NI], F32, tag="dst")
                    nc.gpsimd.ap_gather(
                        dst[:].rearrange("p (n d) -> p n d", d=1),
                        tabs[:, BASE[l]:BASE[l] + ENT[l]].rearrange(
                            "p (n d) -> p n d", d=1),
                        m[:, 0, :],
                        channels=128, num_elems=ENT[l], d=1, num_idxs=NI)

                    # de-interleave: partition q=0 of each core holds the full
                    # gathered stream; fan it out so partition 16k+q gets
                    # stream slice [q*F,(q+1)*F) — 2KB contiguous descriptors.
                    # issue from DVE's queue: its wait on the gather must not
                    # head-of-line-block SP's meta/table prefetches.
                    quad = pq.tile([128, F], F32, tag="quad")
                    nc.vector.dma_start(out=quad[:], in_=dst[::16])

                    qb = quad[:].bitcast(I8).rearrange("p (j r) -> p j r", r=4)
                    v00, v10, v01, v11 = (qb[:, :, c] for c in range(4))
                    fa = m[:, 1, :].bitcast(F16)
                    fb = m[:, 2, :].bitcast(F16)

                    t1 = pt.tile([128, F], F32, tag="t1")
                    t2 = pt.tile([128, F], F32, tag="t2")
                    V = nc.vector
                    # v_f = v00 + fb*(v01 - v00)
                    V.tensor_tensor(out=t1[:], in0=v01, in1=v00, op=sub)
                    V.tensor_tensor(out=t1[:], in0=t1[:], in1=fb, op=mult)
                    V.tensor_tensor(out=t1[:], in0=t1[:], in1=v00, op=add)
                    # v_c = v10 + fb*(v11 - v10)
                    V.tensor_tensor(out=t2[:], in0=v11, in1=v10, op=sub)
                    V.tensor_tensor(out=t2[:], in0=t2[:], in1=fb, op=mult)
                    V.tensor_tensor(out=t2[:], in0=t2[:], in1=v10, op=add)
                    # out = v_f + fa*(v_c - v_f)
                    V.tensor_tensor(out=t2[:], in0=t2[:], in1=t1[:], op=sub)
                    V.tensor_tensor(out=t2[:], in0=t2[:], in1=fa, op=mult)
                    res = pr.tile([128, F], F16, tag="res")
                    V.tensor_tensor(out=res[:], in0=t2[:], in1=t1[:], op=add)

                    nc.sync.dma_start(out=out_t[p, bi, l], in_=res[:])

    nc.compile()
    return nc


# ---------------------------------------------------------------- host tables
def quantize(emb):
    """emb [LEVEL,H,W] f32 -> int8 grids + per-level dequant factors."""
    scl = np.abs(emb).max(axis=(1, 2))
    scl = np.where(scl > 0, scl, 1.0).astype(np.float64)
    q8 = np.clip(np.rint(emb * (127.0 / scl)[:, None, None]),
                 -127, 127).astype(np.int8)
    return q8, (scl / 127.0).astype(np.float64)


def band_row_starts():
    """RS[l][b] = floor(b * BAND_DEG / res_l) = (45*b) >> (l+4), exact."""
    b = np.arange(BANDS, dtype=np.int64)
    return [(45 * b) >> (l + 4) for l in range(LEVEL)]


def build_tables(q8, RS):
    """-> tab [BANDS, TE] f32 (each word = int8 quad [v00,v10,v01,v11])."""
    tab = np.zeros((BANDS, TE, 4), np.int8)
    for l in range(LEVEL):
        cap, wt = CAPS[l], WT[l]
        rows = RS[l][:, None] + np.arange(cap)[None, :]      # [BANDS, cap]
        r0 = np.clip(rows, 0, H_GRID - 1)
        r1 = np.clip(rows + 1, 0, H_GRID - 1)
        g0 = q8[l][r0]                                       # [BANDS, cap, W]
        g1 = q8[l][r1]
        w = np.arange(wt)
        w1 = np.minimum(w + 1, W_GRID - 1)
        ent = np.stack([g0[:, :, w], g1[:, :, w], g0[:, :, w1], g1[:, :, w1]],
                       axis=-1)                              # [BANDS,cap,wt,4]
        tab[:, BASE[l]:BASE[l] + ENT[l]] = ent.reshape(BANDS, ENT[l], 4)
    return np.ascontiguousarray(tab).view('<f4').reshape(BANDS, TE)


# ---------------------------------------------------------------- host points
def point_data(x, RS):
    """Per-point band + per-level (idx int16, fa fp16, fb fp16).

    All index math mirrors the f32 reference exactly; fracs use the
    reference's clamped-floor convention. Returns band [N] and lists of
    per-level arrays."""
    lat = x[:, 0].astype(np.float32)
    lon = x[:, 1].astype(np.float32)
    t32 = np.float32(90.0) - lat
    # band from the same f32 t32 the floors use: floor(t32 * 64 / 45) is
    # computed in f64 where any non-exact case is >= 2^-10/45 away from an
    # integer, so the f64 division can never flip the floor.
    band = np.clip(np.floor(t32.astype(np.float64) * 64.0 / 45.0),
                   0, BANDS - 1).astype(np.int64)
    idxs, fas, fbs = [], [], []
    for l in range(LEVEL):
        r = np.float32(_res(l))
        a = t32 / r
        fl = np.floor(a)
        lat_f = np.clip(fl, 0, H_GRID - 1)
        fa = (a - lat_f).astype(np.float16)
        o = lon / r
        wf = np.clip(np.floor(o), 0, W_GRID - 1)
        fb = (o - wf).astype(np.float16)
        row_local = np.clip(lat_f.astype(np.int64) - RS[l][band],
                            0, CAPS[l] - 1)
        wcol = np.minimum(wf.astype(np.int64), WT[l] - 1)
        idxs.append((row_local * WT[l] + wcol).astype(np.int16))
        fas.append(fa)
        fbs.append(fb)
    return band, idxs, fas, fbs


def slot_assign(band, c_band):
    """slot_global [N]: slot index in [0, BANDS*c_band) per point."""
    order = np.argsort(band, kind="stable")
    counts = np.bincount(band, minlength=BANDS)
    starts = np.zeros(BANDS, np.int64)
    starts[1:] = np.cumsum(counts)[:-1]
    pos_sorted = np.arange(band.size, dtype=np.int64) - starts[band[order]]
    slot_global = np.empty(band.size, np.int64)
    slot_global[order] = band[order] * c_band + pos_sorted
    return slot_global, counts


def _to_lerp_layout(slots, n_batch):
    """[BANDS, c_band] -> [BANDS, nb, 16(q), F(j)]; slot s=(bi*F+j)*16+q."""
    return (slots.reshape(BANDS, n_batch, F, 16).transpose(0, 1, 3, 2))


def _to_idx_layout(slots, n_batch):
    """[BANDS, c_band] -> [BANDS, nb, 16(m), 512(c)]; stream i = q*F+j,
    written at partition m=i%16, col c=i//16."""
    lerp = _to_lerp_layout(slots, n_batch)          # [B, nb, q, j]
    stream = lerp.reshape(BANDS, n_batch, NI)       # i = q*F + j
    return stream.reshape(BANDS, n_batch, F, 16).transpose(0, 1, 3, 2)


# ---------------------------------------------------------------- entry point
_NC_CACHE = {}
LAST_RESULT = None


def kernel(x, embeddings):
    global LAST_RESULT
    from concourse.bass_utils import run_bass_kernel_spmd

    x = np.ascontiguousarray(np.asarray(x), dtype=np.float32)
    emb = np.asarray(embeddings, dtype=np.float32)
    n = x.shape[0]

    q8, deq = quantize(emb)
    RS = band_row_starts()
    tab = build_tables(q8, RS)                      # [BANDS, TE] f32
    band, idxs, fas, fbs = point_data(x, RS)

    counts = np.bincount(band, minlength=BANDS)
    n_batch = 1
    while n_batch * NI < counts.max():
        n_batch += 1
    c_band = n_batch * NI

    if n_batch not in _NC_CACHE:
        _NC_CACHE[n_batch] = build_kernel(n_batch)
    nc = _NC_CACHE[n_batch]

    slot_global, counts = slot_assign(band, c_band)

    # meta [BANDS, nb, LEVEL, 16, 3, F] int16
    meta = np.zeros((BANDS, n_batch, LEVEL, 16, 3, F), np.int16)
    for l in range(LEVEL):
        sl = np.zeros(BANDS * c_band, np.int16)
        sl[slot_global] = idxs[l]
        meta[:, :, l, :, 0, :] = _to_idx_layout(
            sl.reshape(BANDS, c_band), n_batch)
        for ch, v in ((1, fas[l]), (2, fbs[l])):
            sf = np.zeros(BANDS * c_band, np.float16)
            sf[slot_global] = v
            meta[:, :, l, :, ch, :] = _to_lerp_layout(
                sf.reshape(BANDS, c_band), n_batch).view(np.int16)

    # bands -> (rank, pass, core): band = 32r + 8p + k
    tab_r = tab.reshape(N_RANKS, N_PASSES, N_Q7, TE)
    # meta partition dim: [BANDS(r,p,k), nb, L, 16, 3, F]
    #   -> per rank [N_PASSES, nb, LEVEL, 128(k*16+m), 3, F]
    meta_r = (meta.reshape(N_RANKS, N_PASSES, N_Q7, n_batch, LEVEL, 16, 3, F)
              .transpose(0, 1, 3, 4, 2, 5, 6, 7)
              .reshape(N_RANKS, N_PASSES, n_batch, LEVEL, 128, 3, F))

    in_maps = [
        {"tab": np.ascontiguousarray(tab_r[r]),
         "meta": np.ascontiguousarray(meta_r[r])}
        for r in range(N_RANKS)
    ]
    kres = run_bass_kernel_spmd(nc, in_maps, list(range(N_RANKS)))
    LAST_RESULT = kres
    results = kres.results
    res = np.stack([results[r]["out"] for r in range(N_RANKS)])
    # [R, P, nb, L, 128(k,q), F] -> [BANDS, c_band(bi,j,q), LEVEL]
    res = (res.reshape(N_RANKS, N_PASSES, n_batch, LEVEL, N_Q7, 16, F)
           .transpose(0, 1, 4, 2, 6, 5, 3)
           .reshape(BANDS * c_band, LEVEL))

    out = res[slot_global].astype(np.float32) * \
        (np.asarray(deq, np.float32)[None, :])
    assert out.shape == (n, LEVEL)
    return out


# revision 10
# speedup vs baseline: 1.4937x; 1.4937x over previous
"""Trainium2 Bass kernel for multi-level bilinear grid interpolation
(embedding_lookup, nn_COOLCHIC_INTERP_ENC).

Strategy (v2):
  - 8 NeuronCores, data-parallel over query points, sharded spatially by
    latitude into 256 bands (8 ranks x 4 passes x 8 gpsimd cores). Each
    band only touches a handful of grid rows per pyramid level, so each
    band's working set is packed into a per-band table resident in SBUF.
  - Tables store the full bilinear 2x2 quad per (row, col) entry as
    4 x int8 (global per-level symmetric quantization) packed in ONE
    f32 word -> a single d=1 ap_gather index fetches a whole quad.
    Quantization error <= absmax/254 (~0.4%), well inside the 2e-2 gate.
  - Gather indices (int16) and lerp fractions (fp16) are precomputed on
    the host in exactly the layouts the engines want:
      * idx in ap_gather's per-core interleaved stream layout
      * fracs in the lerp layout (partition 16k+q owns stream slice
        [q*F,(q+1)*F) of core k)
    so the gather output de-interleave is ONE SBUF->SBUF DMA with 2KB
    contiguous descriptors (every partition of a core holds the full
    replicated stream; we fan out partition q=0 of each core).
  - DVE does the 9-op bilinear lerp with int8 corner operands, fp32
    intermediates (PSUM), fp16 fracs/result. Host de-quantizes.
"""

import sys

sys.path.insert(0, "/opt/trn_rl_repo")

import numpy as np

from concourse import bacc, bass, mybir
import concourse.tile as tile

# ---------------------------------------------------------------- constants
H_GRID, W_GRID, LEVEL, RES = 721, 1440, 8, 0.25
N_RANKS = 8
N_PASSES = 4
N_Q7 = 8
BANDS = N_RANKS * N_PASSES * N_Q7  # 256
BAND_DEG = 180.0 / BANDS  # 0.703125 (exact binary)
F = 512                   # points per partition per batch
NI = 16 * F               # gather stream length per core (= points/core/batch)

# per-level table geometry: CAP rows x WT cols of quad entries.
# a_l = t32 / res_l is an EXACT power-of-2 scaling of t32 = f32(90 - lat),
# and the band is derived from the same t32 via exact integer arithmetic
# (RS = (45*b) >> (l+4)), so the floor always lands inside the band's row
# window and CAP is exactly the max floor-span per band.
CAPS = [4, 3, 2, 2, 2, 2, 2, 2]
WT = [1440, 720, 360, 180, 90, 45, 23, 12]
ENT = [CAPS[l] * WT[l] for l in range(LEVEL)]
BASE = [sum(ENT[:l]) for l in range(LEVEL)]
TE = sum(ENT)  # 15080 quad entries (f32-packed int8x4) per band

F32 = mybir.dt.float32
F16 = mybir.dt.float16
I16 = mybir.dt.int16
I8 = mybir.dt.int8


def _res(l):
    return RES * (2.0 ** l)


# ---------------------------------------------------------------- device kernel
def build_kernel(n_batch):
    """Per-rank SPMD Bass program. c_band = n_batch * NI points per band."""
    nc = bacc.Bacc(None, target_bir_lowering=False)

    tab_t = nc.declare_dram_parameter("tab", [N_PASSES, N_Q7, TE], F32, False)
    meta_t = nc.declare_dram_parameter(
        "meta", [N_PASSES, n_batch, LEVEL, 128, 3, F], I16, False)
    out_t = nc.declare_dram_parameter(
        "out", [N_PASSES, n_batch, LEVEL, 128, F], F16, True)

    sub = mybir.AluOpType.subtract
    add = mybir.AluOpType.add
    mult = mybir.AluOpType.mult

    from contextlib import ExitStack

    with tile.TileContext(nc) as tc, ExitStack() as es:
        ptab = es.enter_context(tc.tile_pool(name="ptab", bufs=2))
        pdst = es.enter_context(tc.tile_pool(name="pdst", bufs=2))
        pm = es.enter_context(tc.tile_pool(name="pm", bufs=2))
        pq = es.enter_context(tc.tile_pool(name="pq", bufs=2))
        pr = es.enter_context(tc.tile_pool(name="pr", bufs=2))
        pt = es.enter_context(tc.tile_pool(name="pt", bufs=2))

        for p in range(N_PASSES):
            tabs = ptab.tile([128, TE], F32, tag="tabs")
            for q in range(16):
                nc.sync.dma_start(out=tabs[q::16], in_=tab_t[p])

            for bi in range(n_batch):
                for l in range(LEVEL):
                    m = pm.tile([128, 3, F], I16, tag="meta")
                    nc.sync.dma_start(out=m[:], in_=meta_t[p, bi, l])

                    dst = pdst.tile([128, NI], F32, tag="dst")
                    nc.gpsimd.ap_gather(
                        dst[:].rearrange("p (n d) -> p n d", d=1),
                        tabs[:, BASE[l]:BASE[l] + ENT[l]].rearrange(
                            "p (n d) -> p n d", d=1),
                        m[:, 0, :],
                        channels=128, num_elems=ENT[l], d=1, num_idxs=NI)

                    # de-interleave: partition q=0 of each core holds the full
                    # gathered stream; fan it out so partition 16k+q gets
                    # stream slice [q*F,(q+1)*F) — 2KB contiguous descriptors.
                    # issue from Activation queue: its wait on the gather must not
                    # head-of-line-block SP's meta/table prefetches.
                    quad = pq.tile([128, F], F32, tag="quad")
                    nc.scalar.dma_start(out=quad[:], in_=dst[::16])

                    qb = quad[:].bitcast(I8).rearrange("p (j r) -> p j r", r=4)
                    v00, v10, v01, v11 = (qb[:, :, c] for c in range(4))
                    fa = m[:, 1, :].bitcast(F16)
                    fb = m[:, 2, :].bitcast(F16)

                    t1 = pt.tile([128, F], F32, tag="t1")
                    t2 = pt.tile([128, F], F32, tag="t2")
                    V = nc.vector
                    # v_f = v00 + fb*(v01 - v00)
                    V.tensor_tensor(out=t1[:], in0=v01, in1=v00, op=sub)
                    V.tensor_tensor(out=t1[:], in0=t1[:], in1=fb, op=mult)
                    V.tensor_tensor(out=t1[:], in0=t1[:], in1=v00, op=add)
                    # v_c = v10 + fb*(v11 - v10)
                    V.tensor_tensor(out=t2[:], in0=v11, in1=v10, op=sub)
                    V.tensor_tensor(out=t2[:], in0=t2[:], in1=fb, op=mult)
                    V.tensor_tensor(out=t2[:], in0=t2[:], in1=v10, op=add)
                    # out = v_f + fa*(v_c - v_f)
                    V.tensor_tensor(out=t2[:], in0=t2[:], in1=t1[:], op=sub)
                    V.tensor_tensor(out=t2[:], in0=t2[:], in1=fa, op=mult)
                    res = pr.tile([128, F], F16, tag="res")
                    V.tensor_tensor(out=res[:], in0=t2[:], in1=t1[:], op=add)

                    # issue from Activation's (otherwise idle) queue: its
                    # wait on the lerp result must not block SP or DVE.
                    nc.scalar.dma_start(out=out_t[p, bi, l], in_=res[:])

    nc.compile()
    return nc


# ---------------------------------------------------------------- host tables
def quantize(emb):
    """emb [LEVEL,H,W] f32 -> int8 grids + per-level dequant factors."""
    scl = np.abs(emb).max(axis=(1, 2))
    scl = np.where(scl > 0, scl, 1.0).astype(np.float64)
    q8 = np.clip(np.rint(emb * (127.0 / scl)[:, None, None]),
                 -127, 127).astype(np.int8)
    return q8, (scl / 127.0).astype(np.float64)


def band_row_starts():
    """RS[l][b] = floor(b * BAND_DEG / res_l) = (45*b) >> (l+4), exact."""
    b = np.arange(BANDS, dtype=np.int64)
    return [(45 * b) >> (l + 4) for l in range(LEVEL)]


def build_tables(q8, RS):
    """-> tab [BANDS, TE] f32 (each word = int8 quad [v00,v10,v01,v11])."""
    tab = np.zeros((BANDS, TE, 4), np.int8)
    for l in range(LEVEL):
        cap, wt = CAPS[l], WT[l]
        rows = RS[l][:, None] + np.arange(cap)[None, :]      # [BANDS, cap]
        r0 = np.clip(rows, 0, H_GRID - 1)
        r1 = np.clip(rows + 1, 0, H_GRID - 1)
        g0 = q8[l][r0]                                       # [BANDS, cap, W]
        g1 = q8[l][r1]
        w = np.arange(wt)
        w1 = np.minimum(w + 1, W_GRID - 1)
        ent = np.stack([g0[:, :, w], g1[:, :, w], g0[:, :, w1], g1[:, :, w1]],
                       axis=-1)                              # [BANDS,cap,wt,4]
        tab[:, BASE[l]:BASE[l] + ENT[l]] = ent.reshape(BANDS, ENT[l], 4)
    return np.ascontiguousarray(tab).view('<f4').reshape(BANDS, TE)


# ---------------------------------------------------------------- host points
def point_data(x, RS):
    """Per-point band + per-level (idx int16, fa fp16, fb fp16).

    All index math mirrors the f32 reference exactly; fracs use the
    reference's clamped-floor convention. Returns band [N] and lists of
    per-level arrays."""
    lat = x[:, 0].astype(np.float32)
    lon = x[:, 1].astype(np.float32)
    t32 = np.float32(90.0) - lat
    # band from the same f32 t32 the floors use: floor(t32 * 64 / 45) is
    # computed in f64 where any non-exact case is >= 2^-10/45 away from an
    # integer, so the f64 division can never flip the floor.
    band = np.clip(np.floor(t32.astype(np.float64) * 64.0 / 45.0),
                   0, BANDS - 1).astype(np.int64)
    idxs, fas, fbs = [], [], []
    for l in range(LEVEL):
        r = np.float32(_res(l))
        a = t32 / r
        fl = np.floor(a)
        lat_f = np.clip(fl, 0, H_GRID - 1)
        fa = (a - lat_f).astype(np.float16)
        o = lon / r
        wf = np.clip(np.floor(o), 0, W_GRID - 1)
        fb = (o - wf).astype(np.float16)
        row_local = np.clip(lat_f.astype(np.int64) - RS[l][band],
                            0, CAPS[l] - 1)
        wcol = np.minimum(wf.astype(np.int64), WT[l] - 1)
        idxs.append((row_local * WT[l] + wcol).astype(np.int16))
        fas.append(fa)
        fbs.append(fb)
    return band, idxs, fas, fbs


def slot_assign(band, c_band):
    """slot_global [N]: slot index in [0, BANDS*c_band) per point."""
    order = np.argsort(band, kind="stable")
    counts = np.bincount(band, minlength=BANDS)
    starts = np.zeros(BANDS, np.int64)
    starts[1:] = np.cumsum(counts)[:-1]
    pos_sorted = np.arange(band.size, dtype=np.int64) - starts[band[order]]
    slot_global = np.empty(band.size, np.int64)
    slot_global[order] = band[order] * c_band + pos_sorted
    return slot_global, counts


def _to_lerp_layout(slots, n_batch):
    """[BANDS, c_band] -> [BANDS, nb, 16(q), F(j)]; slot s=(bi*F+j)*16+q."""
    return (slots.reshape(BANDS, n_batch, F, 16).transpose(0, 1, 3, 2))


def _to_idx_layout(slots, n_batch):
    """[BANDS, c_band] -> [BANDS, nb, 16(m), 512(c)]; stream i = q*F+j,
    written at partition m=i%16, col c=i//16."""
    lerp = _to_lerp_layout(slots, n_batch)          # [B, nb, q, j]
    stream = lerp.reshape(BANDS, n_batch, NI)       # i = q*F + j
    return stream.reshape(BANDS, n_batch, F, 16).transpose(0, 1, 3, 2)


# ---------------------------------------------------------------- entry point
_NC_CACHE = {}
LAST_RESULT = None


def kernel(x, embeddings):
    global LAST_RESULT
    from concourse.bass_utils import run_bass_kernel_spmd

    x = np.ascontiguousarray(np.asarray(x), dtype=np.float32)
    emb = np.asarray(embeddings, dtype=np.float32)
    n = x.shape[0]

    q8, deq = quantize(emb)
    RS = band_row_starts()
    tab = build_tables(q8, RS)                      # [BANDS, TE] f32
    band, idxs, fas, fbs = point_data(x, RS)

    counts = np.bincount(band, minlength=BANDS)
    n_batch = 1
    while n_batch * NI < counts.max():
        n_batch += 1
    c_band = n_batch * NI

    if n_batch not in _NC_CACHE:
        _NC_CACHE[n_batch] = build_kernel(n_batch)
    nc = _NC_CACHE[n_batch]

    slot_global, counts = slot_assign(band, c_band)

    # meta [BANDS, nb, LEVEL, 16, 3, F] int16
    meta = np.zeros((BANDS, n_batch, LEVEL, 16, 3, F), np.int16)
    for l in range(LEVEL):
        sl = np.zeros(BANDS * c_band, np.int16)
        sl[slot_global] = idxs[l]
        meta[:, :, l, :, 0, :] = _to_idx_layout(
            sl.reshape(BANDS, c_band), n_batch)
        for ch, v in ((1, fas[l]), (2, fbs[l])):
            sf = np.zeros(BANDS * c_band, np.float16)
            sf[slot_global] = v
            meta[:, :, l, :, ch, :] = _to_lerp_layout(
                sf.reshape(BANDS, c_band), n_batch).view(np.int16)

    # bands -> (rank, pass, core): band = 32r + 8p + k
    tab_r = tab.reshape(N_RANKS, N_PASSES, N_Q7, TE)
    # meta partition dim: [BANDS(r,p,k), nb, L, 16, 3, F]
    #   -> per rank [N_PASSES, nb, LEVEL, 128(k*16+m), 3, F]
    meta_r = (meta.reshape(N_RANKS, N_PASSES, N_Q7, n_batch, LEVEL, 16, 3, F)
              .transpose(0, 1, 3, 4, 2, 5, 6, 7)
              .reshape(N_RANKS, N_PASSES, n_batch, LEVEL, 128, 3, F))

    in_maps = [
        {"tab": np.ascontiguousarray(tab_r[r]),
         "meta": np.ascontiguousarray(meta_r[r])}
        for r in range(N_RANKS)
    ]
    kres = run_bass_kernel_spmd(nc, in_maps, list(range(N_RANKS)))
    LAST_RESULT = kres
    results = kres.results
    res = np.stack([results[r]["out"] for r in range(N_RANKS)])
    # [R, P, nb, L, 128(k,q), F] -> [BANDS, c_band(bi,j,q), LEVEL]
    res = (res.reshape(N_RANKS, N_PASSES, n_batch, LEVEL, N_Q7, 16, F)
           .transpose(0, 1, 4, 2, 6, 5, 3)
           .reshape(BANDS * c_band, LEVEL))

    out = res[slot_global].astype(np.float32) * \
        (np.asarray(deq, np.float32)[None, :])
    assert out.shape == (n, LEVEL)
    return out
